# revision 25
# baseline (speedup 1.0000x reference)
"""Trainium2 Bass kernel for DecomposableAttention (B=512, L=256, V=50000, E=300, H=200).

v4: v3 (bf16 PE, 4-batch groups, global-max softmax, zero-row gather,
rank-1 masked-sum correction) + length-class specialization: batches are
host-sorted by (ceil(len1/128), ceil(len2/128)) into classes (1,1), (1,2),
(2,1), (2,2); each group of 4 batches shares a class and only processes the
live 128-position chunks.  Class counts are rounded to multiples of 32 (8
cores x 4 batches) by upgrading leftovers to a superset class, so all cores
run the same program.  The host un-permutes the output rows.
"""
import sys

if '/opt/trn_rl_repo' not in sys.path:
    sys.path.insert(0, '/opt/trn_rl_repo')

import numpy as np

B, L, VOCAB, EMBED, HIDDEN = 512, 256, 50000, 300, 200
NCORES = 8
NB = B // NCORES          # batches per core
GB = 4                    # batches per group
NG = NB // GB             # groups
VZERO = VOCAB             # index of the appended all-zero embedding row

_prog_cache = {}


def make_schedule(len1, len2):
    """Global batch -> per-core permutation + shared group class schedule."""
    ci = np.minimum((len1.astype(np.int64) + 127) // 128, 2)
    cj = np.minimum((len2.astype(np.int64) + 127) // 128, 2)
    buckets = {(1, 1): [], (1, 2): [], (2, 1): [], (2, 2): []}
    for idx in range(len(len1)):
        buckets[(int(ci[idx]), int(cj[idx]))].append(idx)
    unit = NCORES * GB
    for src, dst in [((1, 1), (1, 2)), ((2, 1), (2, 2)), ((1, 2), (2, 2))]:
        keep = len(buckets[src]) - (len(buckets[src]) % unit)
        buckets[dst] = buckets[src][keep:] + buckets[dst]
        buckets[src] = buckets[src][:keep]
    assert len(buckets[(2, 2)]) % unit == 0
    percore = [[] for _ in range(NCORES)]
    classes = [[] for _ in range(NCORES)]   # class per batch slot
    sched = []
    for c in [(1, 1), (1, 2), (2, 1), (2, 2)]:
        lst = buckets[c]
        n = len(lst) // NCORES
        for core in range(NCORES):
            percore[core] += lst[core * n:(core + 1) * n]
            classes[core] += [c] * n
        sched += [c] * (n // GB)
    assert len(sched) == NG
    return percore, classes, sched


def build_program(sched):
    import concourse.bass as bass
    import concourse.bass_isa as bass_isa
    import concourse.bacc as bacc
    import concourse.tile as tile
    import concourse.mybir as mybir
    from concourse.masks import make_identity

    F32 = mybir.dt.float32
    BF16 = mybir.dt.bfloat16
    I32 = mybir.dt.int32
    AX = mybir.AxisListType
    ALU = mybir.AluOpType
    ACTF = mybir.ActivationFunctionType
    P = 128
    EK = [(0, 128), (128, 256), (256, 300)]     # E contraction chunks
    H2 = [(0, 100), (100, 200)]                 # H chunks of 100

    nc = bacc.Bacc("TRN2", num_devices=NCORES)

    emb_d = nc.dram_tensor("emb", [VOCAB + 1, EMBED], BF16, kind="ExternalInput")
    sT_d = nc.dram_tensor("sT", [P, 4 * NB], I32, kind="ExternalInput")
    lmT_d = nc.dram_tensor("lmT", [P, 4 * NB], F32, kind="ExternalInput")
    lenf_d = nc.dram_tensor("lenf", [1, 2 * NB], I32, kind="ExternalInput")
    lmg_d = nc.dram_tensor("lmg", [1, 2 * NB], BF16, kind="ExternalInput")
    # weight matrices are host-padded to 128-column m-blocks so the
    # compiler enables Fast Weight Load (needs exactly 128 weight columns)
    W1a_d = nc.dram_tensor("W1a", [EMBED, 256], BF16, kind="ExternalInput")
    W2a_d = nc.dram_tensor("W2a", [HIDDEN, 256], BF16, kind="ExternalInput")
    W1c_d = nc.dram_tensor("W1c", [2 * EMBED, 256], BF16, kind="ExternalInput")
    W2c_d = nc.dram_tensor("W2c", [HIDDEN, 256], BF16, kind="ExternalInput")
    W1g_d = nc.dram_tensor("W1g", [2 * HIDDEN, 256], BF16, kind="ExternalInput")
    W2g_d = nc.dram_tensor("W2g", [HIDDEN, 2], BF16, kind="ExternalInput")
    b1a_d = nc.dram_tensor("b1a", [100, 2], F32, kind="ExternalInput")
    b2a_d = nc.dram_tensor("b2a", [100, 2], F32, kind="ExternalInput")
    b1c_d = nc.dram_tensor("b1c", [100, 2], F32, kind="ExternalInput")
    b2c_d = nc.dram_tensor("b2c", [100, 2], F32, kind="ExternalInput")
    b1g_d = nc.dram_tensor("b1g", [100, 2], F32, kind="ExternalInput")
    b2g_d = nc.dram_tensor("b2g", [2, 1], F32, kind="ExternalInput")
    out_d = nc.dram_tensor("out", [NB, 2], F32, kind="ExternalOutput")

    with tile.TileContext(nc) as tc:
        import contextlib
        ctx = contextlib.ExitStack()
        with ctx:
            const = ctx.enter_context(tc.tile_pool(name="const", bufs=1))
            gat = ctx.enter_context(tc.tile_pool(name="gat", bufs=2))
            eTp = ctx.enter_context(tc.tile_pool(name="eTp", bufs=2))
            hp = ctx.enter_context(tc.tile_pool(name="hp", bufs=2))
            up = ctx.enter_context(tc.tile_pool(name="up", bufs=2))
            sm = ctx.enter_context(tc.tile_pool(name="sm", bufs=2))
            cmp_ = ctx.enter_context(tc.tile_pool(name="cmp", bufs=2))
            psMLP = ctx.enter_context(tc.tile_pool(name="psMLP", bufs=4, space="PSUM"))
            psSC = ctx.enter_context(tc.tile_pool(name="psSC", bufs=4, space="PSUM"))

            # inputs needed by the first group's gathers -- DMA'd first
            sT_t = const.tile([P, 4 * NB], I32)
            nc.sync.dma_start(sT_t[:], sT_d[:])
            lmT_t = const.tile([P, 4 * NB], F32)
            nc.sync.dma_start(lmT_t[:], lmT_d[:])
            lenf_t = const.tile([1, 2 * NB], I32)
            nc.sync.dma_start(lenf_t[:], lenf_d[:])
            lmg_t = const.tile([1, 2 * NB], BF16)
            nc.sync.dma_start(lmg_t[:], lmg_d[:])

            # ---------------- constants ----------------
            ident_f = const.tile([P, P], F32)
            make_identity(nc, ident_f[:])
            ident = const.tile([P, P], BF16)
            nc.vector.tensor_copy(ident[:], ident_f[:])

            ones_col = const.tile([P, 1], BF16)
            nc.vector.memset(ones_col[:], 1.0)
            ones_row = const.tile([1, P], BF16)
            nc.vector.memset(ones_row[:], 1.0)

            iota_row = const.tile([1, L], I32)
            nc.gpsimd.iota(iota_row[:], pattern=[[1, L]], base=0, channel_multiplier=0)

            # weights (stationary tiles, bf16)
            def wtiles(dram, rows, nm):
                # per-(k-chunk, m-chunk) compact [kw, 128] tiles so the weight
                # AP is contiguous (FWL-safe)
                ts = []
                for i, (k0, k1) in enumerate(rows):
                    pair = []
                    for m in range(2):
                        t = const.tile([k1 - k0, 128], BF16, name=f"{nm}{i}{m}", tag=f"{nm}{i}{m}")
                        nc.sync.dma_start(t[:], dram[k0:k1, m * 128:(m + 1) * 128])
                        pair.append(t)
                    ts.append(pair)
                return ts
            W1a_t = wtiles(W1a_d, EK, "W1a")
            W2a_t = wtiles(W2a_d, H2, "W2a")
            W1ca_t = wtiles(W1c_d, EK, "W1ca")
            W1cb_t = wtiles(W1c_d, [(EMBED + k0, EMBED + k1) for k0, k1 in EK], "W1cb")
            W2c_t = wtiles(W2c_d, H2, "W2c")
            W1g_t = wtiles(W1g_d, [(i * 100, (i + 1) * 100) for i in range(4)], "W1g")
            W2g_t = [const.tile([100, 2], BF16, name=f"W2g{i}", tag=f"W2g{i}") for i in range(2)]
            for i, (k0, k1) in enumerate(H2):
                nc.sync.dma_start(W2g_t[i][:], W2g_d[k0:k1, :])

            def bias_tile(d, nm):
                t = const.tile([d.shape[0], d.shape[1]], F32, name=nm, tag=nm)
                nc.sync.dma_start(t[:], d[:])
                return t
            b1a_t = bias_tile(b1a_d, "b1a")
            b2a_t = bias_tile(b2a_d, "b2a")
            b1c_t = bias_tile(b1c_d, "b1c")
            b2c_t = bias_tile(b2c_d, "b2c")
            b1g_t = bias_tile(b1g_d, "b1g")
            b2g_t = bias_tile(b2g_d, "b2g")

            v_all = [[const.tile([100, NB], F32, name=f"v{s}{m}", tag=f"v{s}{m}")
                      for m in range(2)] for s in range(2)]

            # ---- c0 (compare output at an all-zero input column) + corr ----
            rb1c = const.tile([100, 2], BF16)
            nc.scalar.activation(rb1c[:], b1c_t[:], ACTF.Relu, bias=0.0, scale=1.0)
            c0T = []
            for m, (m0, m1) in enumerate(H2):
                cps = psMLP.tile([P, 1], F32, name="c0_ps", tag="mlp")
                for k in range(2):
                    nc.tensor.matmul(cps[:], W2c_t[k][m][:], rb1c[:, k:k + 1],
                                     start=(k == 0), stop=(k == 1))
                c0m = const.tile([100, 1], F32, name=f"c0{m}", tag=f"c0{m}")
                nc.scalar.activation(c0m[:], cps[:100, :], ACTF.Relu, bias=b2c_t[:, m:m + 1], scale=1.0)
                tps = psSC.tile([1, P], F32, name="c0T_ps", tag="sc")
                nc.tensor.transpose(tps[:, :100], c0m[:], ident_f[:100, :100])
                c0t = const.tile([1, 100], BF16, name=f"c0T{m}", tag=f"c0T{m}")
                nc.vector.tensor_copy(c0t[:], tps[:, :100])
                c0T.append(c0t)
            corr = [[None, None], [None, None]]
            for s in range(2):
                for m in range(2):
                    cps = psMLP.tile([100, NB], F32, name="corr_ps", tag="mlp")
                    nc.tensor.matmul(cps[:], c0T[m][:], lmg_t[:, s * NB:(s + 1) * NB],
                                     start=True, stop=True)
                    ct = const.tile([100, NB], F32, name=f"corr{s}{m}", tag=f"corr{s}{m}")
                    nc.vector.tensor_copy(ct[:], cps[:])
                    corr[s][m] = ct

            def scol(s, c):  # column base in sT/lmT for (sentence, L-chunk)
                return (s * 2 + c) * NB

            def emit_gathers(g):
                b0 = g * GB
                I, J = sched[g]
                CH = (I, J)
                eRg = [[[None, None] for _ in range(2)] for _ in range(GB)]
                for s in range(2):
                    for b4 in range(GB):
                        for c in range(CH[s]):
                            t = gat.tile([P, EMBED], BF16, name=f"eR{b4}{s}{c}", tag=f"eR{b4}{s}{c}")
                            nc.gpsimd.indirect_dma_start(
                                out=t[:], out_offset=None, in_=emb_d[:],
                                in_offset=bass.IndirectOffsetOnAxis(
                                    ap=sT_t[:, scol(s, c) + b0 + b4: scol(s, c) + b0 + b4 + 1],
                                    axis=0),
                            )
                            eRg[b4][s][c] = t
                return eRg

            # ---------------- group loop ----------------
            pending_cmp = []
            eR = emit_gathers(0)
            for g in range(NG):
                b0 = g * GB
                I, J = sched[g]
                CH = (I, J)                    # chunks per sentence
                LS = (128 * I, 128 * J)        # live positions per sentence

                eT_sb = [[None] * 3 for _ in range(2)]
                for s in range(2):
                    for k, (k0, k1) in enumerate(EK):
                        eT_sb[s][k] = eTp.tile([k1 - k0, GB * L], BF16,
                                               name=f"eT{s}{k}", tag=f"eT{s}{k}")

                def emit_tr(s, k, h):
                    # one PSUM tile = 4 transposed [128,128] quarters = 512 cols
                    k0, k1 = EK[k]
                    kw = k1 - k0
                    nch = CH[s]
                    tp = psSC.tile([P, 512], BF16, name="tr_ps", tag="sc")
                    for q in range(4):
                        pos = h * 4 + q
                        b4, c = divmod(pos, nch)
                        nc.tensor.transpose(tp[:kw, q * P:(q + 1) * P],
                                            eR[b4][s][c][:, k0:k1], ident[:])
                    nc.vector.tensor_copy(eT_sb[s][k][:, h * 512:(h + 1) * 512], tp[:kw, :])

                # --- attend L1 (m=0 pass interleaves the transposes) ---
                ha = [[None, None] for _ in range(2)]
                hT = [[None, None] for _ in range(2)]
                for s in range(2):
                    nh = CH[s]                 # halves of 512 cols for this sentence
                    for m, (m0, m1) in enumerate(H2):
                        for h in range(nh):
                            if m == 0:
                                emit_tr(s, 0, h)
                            pp = psMLP.tile([P, 512], F32, name="a1_ps", tag="mlp")
                            for k in range(3):
                                if m == 0 and k + 1 < 3:
                                    emit_tr(s, k + 1, h)
                                nc.tensor.matmul(pp[:], W1a_t[k][m][:],
                                                 eT_sb[s][k][:, h * 512:(h + 1) * 512],
                                                 start=(k == 0), stop=(k == 2))
                            if ha[s][m] is None:
                                ha[s][m] = hp.tile([100, GB * L], BF16, name=f"ha{s}{m}", tag=f"ha{s}{m}")
                            nc.scalar.activation(ha[s][m][:, h * 512:(h + 1) * 512], pp[:100, :],
                                                 ACTF.Relu, bias=b1a_t[:, m:m + 1], scale=1.0)
                # --- attend L2 ---
                for s in range(2):
                    nh = CH[s]
                    for m, (m0, m1) in enumerate(H2):
                        for h in range(nh):
                            qp = psMLP.tile([P, 512], F32, name="a2_ps", tag="mlp")
                            for k2 in range(2):
                                nc.tensor.matmul(qp[:], W2a_t[k2][m][:],
                                                 ha[s][k2][:, h * 512:(h + 1) * 512],
                                                 start=(k2 == 0), stop=(k2 == 1))
                            if hT[s][m] is None:
                                hT[s][m] = hp.tile([100, GB * L], BF16, name=f"hT{s}{m}", tag=f"hT{s}{m}")
                            nc.scalar.activation(hT[s][m][:, h * 512:(h + 1) * 512], qp[:100, :],
                                                 ACTF.Relu, bias=b2a_t[:, m:m + 1], scale=1.0)

                # --- per-batch phase ---
                xsb = [[None] * 3 for _ in range(2)]
                for s in range(2):
                    for k, (k0, k1) in enumerate(EK):
                        xsb[s][k] = eTp.tile([k1 - k0, GB * L], BF16, name=f"x{s}{k}", tag=f"x{s}{k}")

                mr_t = {}
                for b4 in range(GB):
                    for si in range(2):
                        mr = sm.tile([1, L], F32, name=f"mr{b4}{si}", tag=f"mr{b4}{si}")
                        nc.vector.tensor_tensor(
                            mr[:], iota_row[:],
                            lenf_t[:, si * NB + b0 + b4: si * NB + b0 + b4 + 1].to_broadcast([1, L]),
                            op=ALU.is_lt)
                        mr_t[(b4, si)] = mr

                pe_t, pet_t, u_t, bias_t = {}, {}, {}, {}

                def emit_scores(b4):
                    bc = (b4 * LS[0], b4 * LS[1])
                    pe = psSC.tile([P, 512], F32, name="pe", tag="sc")
                    pet = psSC.tile([P, 512], F32, name="pet", tag="sc")
                    for ic in range(I):
                        for m in range(2):
                            nc.tensor.matmul(pe[:, ic * LS[1]:(ic + 1) * LS[1]],
                                             hT[0][m][:, bc[0] + ic * P: bc[0] + (ic + 1) * P],
                                             hT[1][m][:, bc[1]:bc[1] + LS[1]],
                                             start=(m == 0), stop=(m == 1))
                    for jc in range(J):
                        for m in range(2):
                            nc.tensor.matmul(pet[:, jc * LS[0]:(jc + 1) * LS[0]],
                                             hT[1][m][:, bc[1] + jc * P: bc[1] + (jc + 1) * P],
                                             hT[0][m][:, bc[0]:bc[0] + LS[0]],
                                             start=(m == 0), stop=(m == 1))
                    pe_t[b4], pet_t[b4] = pe, pet

                def emit_softmax(b4):
                    b = b0 + b4
                    pe, pet = pe_t[b4], pet_t[b4]
                    mx = sm.tile([P, 1], F32, name="mx", tag="mx")
                    nc.vector.tensor_reduce(mx[:], pe[:, :I * LS[1]], axis=AX.X, op=ALU.max)
                    Gb = sm.tile([P, 1], F32, name="Gb", tag="Gb")
                    nc.gpsimd.partition_all_reduce(Gb[:], mx[:], channels=P,
                                                   reduce_op=bass_isa.ReduceOp.max)
                    bias_t[b4] = {}
                    for d in range(2):
                        for c in range(CH[d]):
                            bt = sm.tile([P, 1], F32, name=f"bx{d}{c}", tag=f"bx{d}{c}")
                            nc.vector.tensor_tensor(
                                bt[:], lmT_t[:, scol(d, c) + b: scol(d, c) + b + 1],
                                Gb[:], op=ALU.subtract)
                            bias_t[b4][(d, c)] = bt
                    u_t[b4] = {}
                    for d, src in ((0, pe), (1, pet)):
                        w = LS[1 - d]
                        for c in range(CH[d]):
                            ut = up.tile([P, L], BF16, name=f"u{b4}{d}{c}", tag=f"u{b4}{d}{c}")
                            nc.scalar.activation(ut[:, :w], src[:, c * w:(c + 1) * w], ACTF.Exp,
                                                 bias=bias_t[b4][(d, c)][:], scale=1.0)
                            u_t[b4][(d, c)] = ut

                def emit_attn(b4):
                    bc = (b4 * LS[0], b4 * LS[1])
                    u = u_t[b4]
                    den = psSC.tile([1, 512], F32, name="den", tag="sc")
                    for d in range(2):
                        w = LS[1 - d]
                        for c in range(CH[d]):
                            nc.tensor.matmul(den[:, d * 256:d * 256 + w], ones_col[:],
                                             u[(d, c)][:, :w],
                                             start=(c == 0), stop=(c == CH[d] - 1))
                    rc = sm.tile([1, 512], F32, name="rc", tag="rc")
                    nc.vector.reciprocal_approx_fast(rc[:, :LS[1]], den[:, :LS[1]])
                    nc.vector.reciprocal_approx_fast(rc[:, 256:256 + LS[0]],
                                                     den[:, 256:256 + LS[0]])
                    rm_t = []
                    for d in range(2):
                        w = LS[1 - d]
                        rm = sm.tile([1, L], BF16, name=f"rm{d}", tag=f"rm{d}")
                        nc.vector.tensor_tensor(rm[:, :w], rc[:, d * 256:d * 256 + w],
                                                mr_t[(b4, 1 - d)][:, :w], op=ALU.mult)
                        rm_t.append(rm)

                    def attn_mms(d):
                        w = LS[1 - d]
                        aps = []
                        for k, (k0, k1) in enumerate(EK):
                            kw = k1 - k0
                            ap_ = psMLP.tile([P, 512], F32, name="attn_ps", tag="mlp")
                            for c in range(CH[d]):
                                nc.tensor.matmul(ap_[:kw, :w], eR[b4][d][c][:, k0:k1],
                                                 u[(d, c)][:, :w],
                                                 start=(c == 0), stop=(c == CH[d] - 1))
                            aps.append(ap_)
                        return aps

                    def xt_mults(d, aps):
                        w = LS[1 - d]
                        for k, (k0, k1) in enumerate(EK):
                            nc.vector.tensor_tensor(xsb[1 - d][k][:, bc[1 - d]:bc[1 - d] + w],
                                                    aps[k][:k1 - k0, :w],
                                                    Rs[:k1 - k0, d * 256:d * 256 + w],
                                                    op=ALU.mult)

                    # d=0 attention matmuls run between den and the R matmul so
                    # the PE covers the den->recip->rm DVE latency
                    aps0 = attn_mms(0)
                    Rp = psMLP.tile([P, 512], F32, name="R_ps", tag="mlp")
                    for d in range(2):
                        w = LS[1 - d]
                        nc.tensor.matmul(Rp[:, d * 256:d * 256 + w], ones_row[:],
                                         rm_t[d][:, :w], start=True, stop=True)
                    Rs = sm.tile([P, 512], BF16, name="Rs", tag="Rs")
                    nc.scalar.activation(Rs[:], Rp[:], ACTF.Identity, bias=0.0, scale=1.0)
                    xt_mults(0, aps0)
                    xt_mults(1, attn_mms(1))

                r1 = [[None, None] for _ in range(2)]
                for s in range(2):
                    for m in range(2):
                        r1[s][m] = cmp_.tile([100, GB * L], BF16, name=f"r1{s}{m}", tag=f"r1{s}{m}")

                def emit_compare(s, h, _eT=eT_sb, _x=xsb, _r1=r1, _b0=b0, _LS=LS):
                    # compare L1+L2 for sentence s, half h.  Per-group state is
                    # bound via default args so a deferred call (executed during
                    # the NEXT group's batch phase) still sees this group's tiles.
                    for m, (m0, m1) in enumerate(H2):
                        cp = psMLP.tile([P, 512], F32, name="c1_ps", tag="mlp")
                        for k in range(3):
                            nc.tensor.matmul(cp[:], W1ca_t[k][m][:],
                                             _eT[s][k][:, h * 512:(h + 1) * 512],
                                             start=(k == 0), stop=False)
                        for k in range(3):
                            nc.tensor.matmul(cp[:], W1cb_t[k][m][:],
                                             _x[s][k][:, h * 512:(h + 1) * 512],
                                             start=False, stop=(k == 2))
                        reg = _r1[s][m][:, h * 512:(h + 1) * 512]
                        nc.scalar.activation(reg, cp[:100, :], ACTF.Relu,
                                             bias=b1c_t[:, m:m + 1], scale=1.0)
                    segs = 512 // _LS[s]
                    for m, (m0, m1) in enumerate(H2):
                        cq = psMLP.tile([P, 512], F32, name="c2_ps", tag="mlp")
                        for k2 in range(2):
                            nc.tensor.matmul(cq[:], W2c_t[k2][m][:],
                                             _r1[s][k2][:, h * 512:(h + 1) * 512],
                                             start=(k2 == 0), stop=(k2 == 1))
                        for q in range(segs):
                            b4 = h * segs + q
                            scr = cmp_.tile([100, L], BF16, name="c2scr", tag=f"c2scr{s}")
                            nc.scalar.activation(
                                scr[:, :_LS[s]], cq[:100, q * _LS[s]:(q + 1) * _LS[s]], ACTF.Relu,
                                bias=b2c_t[:, m:m + 1], scale=1.0,
                                accum_out=v_all[s][m][:, _b0 + b4:_b0 + b4 + 1])

                emit_scores(0)
                emit_softmax(0)
                emit_scores(1)
                for fn in pending_cmp:
                    fn()
                pending_cmp = []
                emit_attn(0)
                emit_softmax(1)
                emit_scores(2)
                emit_attn(1)
                emit_softmax(2)
                # halves done after batches 0,1: sentence s half h covers
                # batches [h*512//LS[s], ...); emit compare for halves fully
                # covered by batches 0..1
                for s in range(2):
                    if CH[s] == 2:
                        emit_compare(s, 0)
                emit_scores(3)
                emit_attn(2)
                emit_softmax(3)
                emit_attn(3)
                if g + 1 < NG:
                    eR_next = emit_gathers(g + 1)
                else:
                    eR_next = None
                emit_compare(0, 1 if CH[0] == 2 else 0)
                h1 = 1 if CH[1] == 2 else 0
                if g + 1 < NG:
                    pending_cmp = [lambda f=emit_compare, hh=h1: f(1, hh)]
                else:
                    emit_compare(1, h1)
                eR = eR_next

            # ---------------- aggregate ----------------
            vb = []
            for s in range(2):
                for m in range(2):
                    t = const.tile([100, NB], BF16, name=f"vb{s}{m}", tag=f"vb{s}{m}")
                    nc.vector.tensor_tensor(t[:], v_all[s][m][:], corr[s][m][:], op=ALU.subtract)
                    vb.append(t)
            g1 = []
            for m, (m0, m1) in enumerate(H2):
                gp = psMLP.tile([P, NB], F32, name="g_ps", tag="mlp")
                for k in range(4):
                    nc.tensor.matmul(gp[:], W1g_t[k][m][:], vb[k][:],
                                     start=(k == 0), stop=(k == 3))
                gt = const.tile([100, NB], BF16, name=f"g1{m}", tag=f"g1{m}")
                nc.scalar.activation(gt[:], gp[:100, :], ACTF.Relu, bias=b1g_t[:, m:m + 1], scale=1.0)
                g1.append(gt)
            op = psMLP.tile([2, NB], F32, name="o_ps", tag="mlp")
            for k2 in range(2):
                nc.tensor.matmul(op[:], W2g_t[k2][:], g1[k2][:],
                                 start=(k2 == 0), stop=(k2 == 1))
            osb = const.tile([2, NB], F32, name="osb", tag="osb")
            nc.scalar.activation(osb[:], op[:], ACTF.Identity, bias=b2g_t[:], scale=1.0)
            nc.sync.dma_start(out_d[:].rearrange("b o -> o b"), osb[:])

    nc.compile()
    return nc


def _shard_inputs(inputs, percore, classes):
    import ml_dtypes
    BF = ml_dtypes.bfloat16
    f = np.ascontiguousarray

    emb = np.zeros((VOCAB + 1, EMBED), dtype=BF)
    emb[:VOCAB] = inputs['emb'].astype(BF)

    def padw(w):  # [K, 200] -> [K, 256] with m-chunk m at cols m*128:m*128+100
        out = np.zeros((w.shape[0], 256), dtype=BF)
        out[:, 0:100] = w[:, 0:100].astype(BF)
        out[:, 128:228] = w[:, 100:200].astype(BF)
        return f(out)
    Wb = {k: padw(inputs[k]) for k in ('W1a', 'W2a', 'W1c', 'W2c', 'W1g')}
    Wb['W2g'] = f(inputs['W2g'].astype(BF))
    bias = {k: f(inputs[k].astype(np.float32).reshape(2, 100).T)
            for k in ('b1a', 'b2a', 'b1c', 'b2c', 'b1g')}
    b2g = f(inputs['b2g'].astype(np.float32).reshape(2, 1))

    pos = np.arange(L)
    maps = []
    for cid in range(NCORES):
        idx = np.array(percore[cid], dtype=np.int64)
        cls = classes[cid]
        s = [inputs['s1'][idx].astype(np.int32), inputs['s2'][idx].astype(np.int32)]
        ln = [inputs['len1'][idx].astype(np.int32), inputs['len2'][idx].astype(np.int32)]
        chunks = np.array([[c[0] for c in cls], [c[1] for c in cls]], dtype=np.int32)  # [2, NB]
        sT = np.zeros((128, 4 * NB), dtype=np.int32)
        lmT = np.zeros((128, 4 * NB), dtype=np.float32)
        lenf = np.zeros((1, 2 * NB), dtype=np.int32)
        lmg = np.zeros((1, 2 * NB), dtype=BF)
        for si in range(2):
            valid = pos[None, :] < ln[si][:, None]          # [NB, L]
            sm_ = np.where(valid, s[si], VZERO)
            for c in range(2):
                col = (si * 2 + c) * NB
                sT[:, col:col + NB] = sm_[:, c * 128:(c + 1) * 128].T
                lmT[:, col:col + NB] = np.where(valid[:, c * 128:(c + 1) * 128], 0.0, -30000.0).T
            lenf[0, si * NB:(si + 1) * NB] = ln[si]
            lmg[0, si * NB:(si + 1) * NB] = (128 * chunks[si] - ln[si]).astype(BF)
        maps.append(dict(
            emb=emb, sT=f(sT), lmT=f(lmT), lenf=f(lenf), lmg=f(lmg),
            W1a=Wb['W1a'], W2a=Wb['W2a'], W1c=Wb['W1c'], W2c=Wb['W2c'],
            W1g=Wb['W1g'], W2g=Wb['W2g'],
            b1a=bias['b1a'], b2a=bias['b2a'], b1c=bias['b1c'], b2c=bias['b2c'],
            b1g=bias['b1g'], b2g=b2g,
        ))
    return maps


def kernel(**inputs):
    from concourse.bass_utils import run_bass_kernel_spmd
    len1 = np.asarray(inputs['len1'])
    len2 = np.asarray(inputs['len2'])
    percore, classes, sched = make_schedule(len1, len2)
    key = tuple(sched)
    if key not in _prog_cache:
        _prog_cache[key] = build_program(sched)
        _prog_cache['last'] = (percore, classes, sched)
    nc = _prog_cache[key]
    in_maps = _shard_inputs(inputs, percore, classes)
    res = run_bass_kernel_spmd(nc, in_maps, core_ids=list(range(NCORES)))
    rows = np.concatenate([res.results[c]["out"] for c in range(NCORES)], axis=0)
    perm = np.concatenate([np.array(p, dtype=np.int64) for p in percore])
    out = np.empty((B, 2), dtype=np.float32)
    out[perm] = rows.astype(np.float32)
    return out


# revision 27
# speedup vs baseline: 1.1128x; 1.1128x over previous
"""Trainium2 Bass kernel for DecomposableAttention (B=512, L=256, V=50000, E=300, H=200).

v4: v3 (bf16 PE, 4-batch groups, global-max softmax, zero-row gather,
rank-1 masked-sum correction) + length-class specialization: batches are
host-sorted by (ceil(len1/128), ceil(len2/128)) into classes (1,1), (1,2),
(2,1), (2,2); each group of 4 batches shares a class and only processes the
live 128-position chunks.  Class counts are rounded to multiples of 32 (8
cores x 4 batches) by upgrading leftovers to a superset class, so all cores
run the same program.  The host un-permutes the output rows.
"""
import sys

if '/opt/trn_rl_repo' not in sys.path:
    sys.path.insert(0, '/opt/trn_rl_repo')

import numpy as np

B, L, VOCAB, EMBED, HIDDEN = 512, 256, 50000, 300, 200
NCORES = 8
NB = B // NCORES          # batches per core
GB = 4                    # batches per group
NG = NB // GB             # groups
VZERO = VOCAB             # index of the appended all-zero embedding row

_prog_cache = {}


def make_schedule(len1, len2):
    """Global batch -> per-core permutation + shared group class schedule."""
    ci = np.minimum((len1.astype(np.int64) + 127) // 128, 2)
    cj = np.minimum((len2.astype(np.int64) + 127) // 128, 2)
    buckets = {(1, 1): [], (1, 2): [], (2, 1): [], (2, 2): []}
    for idx in range(len(len1)):
        buckets[(int(ci[idx]), int(cj[idx]))].append(idx)
    unit = NCORES * GB
    for src, dst in [((1, 1), (1, 2)), ((2, 1), (2, 2)), ((1, 2), (2, 2))]:
        keep = len(buckets[src]) - (len(buckets[src]) % unit)
        buckets[dst] = buckets[src][keep:] + buckets[dst]
        buckets[src] = buckets[src][:keep]
    assert len(buckets[(2, 2)]) % unit == 0
    percore = [[] for _ in range(NCORES)]
    classes = [[] for _ in range(NCORES)]   # class per batch slot
    sched = []
    for c in [(1, 1), (1, 2), (2, 1), (2, 2)]:
        lst = buckets[c]
        n = len(lst) // NCORES
        for core in range(NCORES):
            percore[core] += lst[core * n:(core + 1) * n]
            classes[core] += [c] * n
        sched += [c] * (n // GB)
    assert len(sched) == NG
    return percore, classes, sched


def build_program(sched):
    import concourse.bass as bass
    import concourse.bass_isa as bass_isa
    import concourse.bacc as bacc
    import concourse.tile as tile
    import concourse.mybir as mybir
    from concourse.masks import make_identity

    F32 = mybir.dt.float32
    BF16 = mybir.dt.bfloat16
    I32 = mybir.dt.int32
    AX = mybir.AxisListType
    ALU = mybir.AluOpType
    ACTF = mybir.ActivationFunctionType
    P = 128
    EK = [(0, 128), (128, 256), (256, 300)]     # E contraction chunks
    H2 = [(0, 100), (100, 200)]                 # H chunks of 100

    nc = bacc.Bacc("TRN2", num_devices=NCORES)

    emb_d = nc.dram_tensor("emb", [VOCAB + 1, EMBED], BF16, kind="ExternalInput")
    sT_d = nc.dram_tensor("sT", [P, 4 * NB], I32, kind="ExternalInput")
    lmT_d = nc.dram_tensor("lmT", [P, 4 * NB], F32, kind="ExternalInput")
    lenf_d = nc.dram_tensor("lenf", [1, 2 * NB], I32, kind="ExternalInput")
    lmg_d = nc.dram_tensor("lmg", [1, 2 * NB], BF16, kind="ExternalInput")
    # weight matrices are host-padded to 128-column m-blocks so the
    # compiler enables Fast Weight Load (needs exactly 128 weight columns)
    W1a_d = nc.dram_tensor("W1a", [EMBED, 256], BF16, kind="ExternalInput")
    W2a_d = nc.dram_tensor("W2a", [HIDDEN, 256], BF16, kind="ExternalInput")
    W1c_d = nc.dram_tensor("W1c", [2 * EMBED, 256], BF16, kind="ExternalInput")
    W2c_d = nc.dram_tensor("W2c", [HIDDEN, 256], BF16, kind="ExternalInput")
    W1g_d = nc.dram_tensor("W1g", [2 * HIDDEN, 256], BF16, kind="ExternalInput")
    W2g_d = nc.dram_tensor("W2g", [HIDDEN, 2], BF16, kind="ExternalInput")
    b1a_d = nc.dram_tensor("b1a", [100, 2], F32, kind="ExternalInput")
    b2a_d = nc.dram_tensor("b2a", [100, 2], F32, kind="ExternalInput")
    b1c_d = nc.dram_tensor("b1c", [100, 2], F32, kind="ExternalInput")
    b2c_d = nc.dram_tensor("b2c", [100, 2], F32, kind="ExternalInput")
    b1g_d = nc.dram_tensor("b1g", [100, 2], F32, kind="ExternalInput")
    b2g_d = nc.dram_tensor("b2g", [2, 1], F32, kind="ExternalInput")
    out_d = nc.dram_tensor("out", [NB, 2], F32, kind="ExternalOutput")

    with tile.TileContext(nc) as tc:
        import contextlib
        ctx = contextlib.ExitStack()
        with ctx:
            const = ctx.enter_context(tc.tile_pool(name="const", bufs=1))
            gat = ctx.enter_context(tc.tile_pool(name="gat", bufs=2))
            eTp = ctx.enter_context(tc.tile_pool(name="eTp", bufs=2))
            hp = ctx.enter_context(tc.tile_pool(name="hp", bufs=2))
            up = ctx.enter_context(tc.tile_pool(name="up", bufs=2))
            sm = ctx.enter_context(tc.tile_pool(name="sm", bufs=2))
            cmp_ = ctx.enter_context(tc.tile_pool(name="cmp", bufs=2))
            psMLP = ctx.enter_context(tc.tile_pool(name="psMLP", bufs=4, space="PSUM"))
            psSC = ctx.enter_context(tc.tile_pool(name="psSC", bufs=4, space="PSUM"))

            # inputs needed by the first group's gathers -- DMA'd first
            sT_t = const.tile([P, 4 * NB], I32)
            nc.sync.dma_start(sT_t[:], sT_d[:])
            lmT_t = const.tile([P, 4 * NB], F32)
            nc.sync.dma_start(lmT_t[:], lmT_d[:])
            lenf_t = const.tile([1, 2 * NB], I32)
            nc.sync.dma_start(lenf_t[:], lenf_d[:])
            lmg_t = const.tile([1, 2 * NB], BF16)
            nc.sync.dma_start(lmg_t[:], lmg_d[:])

            # ---------------- constants ----------------
            ident_f = const.tile([P, P], F32)
            make_identity(nc, ident_f[:])
            ident = const.tile([P, P], BF16)
            nc.vector.tensor_copy(ident[:], ident_f[:])

            ones_col = const.tile([P, 1], BF16)
            nc.vector.memset(ones_col[:], 1.0)
            ones_row = const.tile([1, P], BF16)
            nc.vector.memset(ones_row[:], 1.0)

            iota_row = const.tile([1, L], I32)
            nc.gpsimd.iota(iota_row[:], pattern=[[1, L]], base=0, channel_multiplier=0)

            # weights (stationary tiles, bf16)
            def wtiles(dram, rows, nm):
                # per-(k-chunk, m-chunk) compact [kw, 128] tiles so the weight
                # AP is contiguous (FWL-safe)
                ts = []
                for i, (k0, k1) in enumerate(rows):
                    pair = []
                    for m in range(2):
                        t = const.tile([k1 - k0, 128], BF16, name=f"{nm}{i}{m}", tag=f"{nm}{i}{m}")
                        nc.sync.dma_start(t[:], dram[k0:k1, m * 128:(m + 1) * 128])
                        pair.append(t)
                    ts.append(pair)
                return ts
            W1a_t = wtiles(W1a_d, EK, "W1a")
            W2a_t = wtiles(W2a_d, H2, "W2a")
            W1ca_t = wtiles(W1c_d, EK, "W1ca")
            W1cb_t = wtiles(W1c_d, [(EMBED + k0, EMBED + k1) for k0, k1 in EK], "W1cb")
            W2c_t = wtiles(W2c_d, H2, "W2c")
            W1g_t = wtiles(W1g_d, [(i * 100, (i + 1) * 100) for i in range(4)], "W1g")
            W2g_t = [const.tile([100, 2], BF16, name=f"W2g{i}", tag=f"W2g{i}") for i in range(2)]
            for i, (k0, k1) in enumerate(H2):
                nc.sync.dma_start(W2g_t[i][:], W2g_d[k0:k1, :])

            def bias_tile(d, nm):
                t = const.tile([d.shape[0], d.shape[1]], F32, name=nm, tag=nm)
                nc.sync.dma_start(t[:], d[:])
                return t
            b1a_t = bias_tile(b1a_d, "b1a")
            b2a_t = bias_tile(b2a_d, "b2a")
            b1c_t = bias_tile(b1c_d, "b1c")
            b2c_t = bias_tile(b2c_d, "b2c")
            b1g_t = bias_tile(b1g_d, "b1g")
            b2g_t = bias_tile(b2g_d, "b2g")

            v_all = [[const.tile([100, NB], F32, name=f"v{s}{m}", tag=f"v{s}{m}")
                      for m in range(2)] for s in range(2)]

            # ---- c0 (compare output at an all-zero input column) + corr ----
            rb1c = const.tile([100, 2], BF16)
            nc.scalar.activation(rb1c[:], b1c_t[:], ACTF.Relu, bias=0.0, scale=1.0)
            c0T = []
            for m, (m0, m1) in enumerate(H2):
                cps = psMLP.tile([P, 1], F32, name="c0_ps", tag="mlp")
                for k in range(2):
                    nc.tensor.matmul(cps[:], W2c_t[k][m][:], rb1c[:, k:k + 1],
                                     start=(k == 0), stop=(k == 1))
                c0m = const.tile([100, 1], F32, name=f"c0{m}", tag=f"c0{m}")
                nc.scalar.activation(c0m[:], cps[:100, :], ACTF.Relu, bias=b2c_t[:, m:m + 1], scale=1.0)
                tps = psSC.tile([1, P], F32, name="c0T_ps", tag="sc")
                nc.tensor.transpose(tps[:, :100], c0m[:], ident_f[:100, :100])
                c0t = const.tile([1, 100], BF16, name=f"c0T{m}", tag=f"c0T{m}")
                nc.vector.tensor_copy(c0t[:], tps[:, :100])
                c0T.append(c0t)
            corr = [[None, None], [None, None]]
            for s in range(2):
                for m in range(2):
                    cps = psMLP.tile([100, NB], F32, name="corr_ps", tag="mlp")
                    nc.tensor.matmul(cps[:], c0T[m][:], lmg_t[:, s * NB:(s + 1) * NB],
                                     start=True, stop=True)
                    ct = const.tile([100, NB], F32, name=f"corr{s}{m}", tag=f"corr{s}{m}")
                    nc.vector.tensor_copy(ct[:], cps[:])
                    corr[s][m] = ct

            def scol(s, c):  # column base in sT/lmT for (sentence, L-chunk)
                return (s * 2 + c) * NB

            def emit_gathers(g):
                b0 = g * GB
                I, J = sched[g]
                CH = (I, J)
                eRg = [[[None, None] for _ in range(2)] for _ in range(GB)]
                for s in range(2):
                    for b4 in range(GB):
                        for c in range(CH[s]):
                            t = gat.tile([P, EMBED], BF16, name=f"eR{b4}{s}{c}", tag=f"eR{b4}{s}{c}")
                            nc.gpsimd.indirect_dma_start(
                                out=t[:], out_offset=None, in_=emb_d[:],
                                in_offset=bass.IndirectOffsetOnAxis(
                                    ap=sT_t[:, scol(s, c) + b0 + b4: scol(s, c) + b0 + b4 + 1],
                                    axis=0),
                            )
                            eRg[b4][s][c] = t
                return eRg

            # ---------------- group loop ----------------
            pending_cmp = []
            eR = emit_gathers(0)
            for g in range(NG):
                b0 = g * GB
                I, J = sched[g]
                CH = (I, J)                    # chunks per sentence
                LS = (128 * I, 128 * J)        # live positions per sentence

                eT_sb = [[None] * 3 for _ in range(2)]
                for s in range(2):
                    for k, (k0, k1) in enumerate(EK):
                        eT_sb[s][k] = eTp.tile([k1 - k0, GB * L], BF16,
                                               name=f"eT{s}{k}", tag=f"eT{s}{k}")

                def emit_tr(s, k, h):
                    # one PSUM tile = 4 transposed [128,128] quarters = 512 cols
                    k0, k1 = EK[k]
                    kw = k1 - k0
                    nch = CH[s]
                    tp = psSC.tile([P, 512], BF16, name="tr_ps", tag="sc")
                    for q in range(4):
                        pos = h * 4 + q
                        b4, c = divmod(pos, nch)
                        nc.tensor.transpose(tp[:kw, q * P:(q + 1) * P],
                                            eR[b4][s][c][:, k0:k1], ident[:])
                    nc.vector.tensor_copy(eT_sb[s][k][:, h * 512:(h + 1) * 512], tp[:kw, :])

                # --- attend L1 (m=0 pass interleaves the transposes) ---
                ha = [[None, None] for _ in range(2)]
                hT = [[None, None] for _ in range(2)]
                for s in range(2):
                    nh = CH[s]                 # halves of 512 cols for this sentence
                    for m, (m0, m1) in enumerate(H2):
                        for h in range(nh):
                            if m == 0:
                                emit_tr(s, 0, h)
                            pp = psMLP.tile([P, 512], F32, name="a1_ps", tag="mlp")
                            for k in range(3):
                                if m == 0 and k + 1 < 3:
                                    emit_tr(s, k + 1, h)
                                nc.tensor.matmul(pp[:], W1a_t[k][m][:],
                                                 eT_sb[s][k][:, h * 512:(h + 1) * 512],
                                                 start=(k == 0), stop=(k == 2))
                            if ha[s][m] is None:
                                ha[s][m] = hp.tile([100, GB * L], BF16, name=f"ha{s}{m}", tag=f"ha{s}{m}")
                            nc.scalar.activation(ha[s][m][:, h * 512:(h + 1) * 512], pp[:100, :],
                                                 ACTF.Relu, bias=b1a_t[:, m:m + 1], scale=1.0)
                # --- attend L2 ---
                for s in range(2):
                    nh = CH[s]
                    for m, (m0, m1) in enumerate(H2):
                        for h in range(nh):
                            qp = psMLP.tile([P, 512], F32, name="a2_ps", tag="mlp")
                            for k2 in range(2):
                                nc.tensor.matmul(qp[:], W2a_t[k2][m][:],
                                                 ha[s][k2][:, h * 512:(h + 1) * 512],
                                                 start=(k2 == 0), stop=(k2 == 1))
                            if hT[s][m] is None:
                                hT[s][m] = hp.tile([100, GB * L], BF16, name=f"hT{s}{m}", tag=f"hT{s}{m}")
                            nc.scalar.activation(hT[s][m][:, h * 512:(h + 1) * 512], qp[:100, :],
                                                 ACTF.Relu, bias=b2a_t[:, m:m + 1], scale=1.0)

                # --- per-batch phase ---
                xsb = [[None] * 3 for _ in range(2)]
                for s in range(2):
                    for k, (k0, k1) in enumerate(EK):
                        xsb[s][k] = eTp.tile([k1 - k0, GB * L], BF16, name=f"x{s}{k}", tag=f"x{s}{k}")

                mr_t = {}
                for b4 in range(GB):
                    for si in range(2):
                        mr = sm.tile([1, L], F32, name=f"mr{b4}{si}", tag=f"mr{b4}{si}")
                        nc.vector.tensor_tensor(
                            mr[:], iota_row[:],
                            lenf_t[:, si * NB + b0 + b4: si * NB + b0 + b4 + 1].to_broadcast([1, L]),
                            op=ALU.is_lt)
                        mr_t[(b4, si)] = mr

                pe_t, pet_t, u_t, bias_t = {}, {}, {}, {}

                def emit_scores(b4):
                    bc = (b4 * LS[0], b4 * LS[1])
                    pe = psSC.tile([P, 512], F32, name="pe", tag="sc")
                    pet = psSC.tile([P, 512], F32, name="pet", tag="sc")
                    for ic in range(I):
                        for m in range(2):
                            nc.tensor.matmul(pe[:, ic * LS[1]:(ic + 1) * LS[1]],
                                             hT[0][m][:, bc[0] + ic * P: bc[0] + (ic + 1) * P],
                                             hT[1][m][:, bc[1]:bc[1] + LS[1]],
                                             start=(m == 0), stop=(m == 1))
                    for jc in range(J):
                        for m in range(2):
                            nc.tensor.matmul(pet[:, jc * LS[0]:(jc + 1) * LS[0]],
                                             hT[1][m][:, bc[1] + jc * P: bc[1] + (jc + 1) * P],
                                             hT[0][m][:, bc[0]:bc[0] + LS[0]],
                                             start=(m == 0), stop=(m == 1))
                    pe_t[b4], pet_t[b4] = pe, pet

                def emit_softmax(b4):
                    b = b0 + b4
                    pe, pet = pe_t[b4], pet_t[b4]
                    mx = sm.tile([P, 1], F32, name="mx", tag="mx")
                    nc.vector.tensor_reduce(mx[:], pe[:, :I * LS[1]], axis=AX.X, op=ALU.max)
                    Gb = sm.tile([P, 1], F32, name="Gb", tag="Gb")
                    nc.gpsimd.partition_all_reduce(Gb[:], mx[:], channels=P,
                                                   reduce_op=bass_isa.ReduceOp.max)
                    bias_t[b4] = {}
                    for d in range(2):
                        for c in range(CH[d]):
                            bt = sm.tile([P, 1], F32, name=f"bx{d}{c}", tag=f"bx{d}{c}")
                            nc.vector.tensor_tensor(
                                bt[:], lmT_t[:, scol(d, c) + b: scol(d, c) + b + 1],
                                Gb[:], op=ALU.subtract)
                            bias_t[b4][(d, c)] = bt
                    u_t[b4] = {}
                    for d, src in ((0, pe), (1, pet)):
                        w = LS[1 - d]
                        for c in range(CH[d]):
                            ut = up.tile([P, L], BF16, name=f"u{b4}{d}{c}", tag=f"u{b4}{d}{c}")
                            nc.scalar.activation(ut[:, :w], src[:, c * w:(c + 1) * w], ACTF.Exp,
                                                 bias=bias_t[b4][(d, c)][:], scale=1.0)
                            u_t[b4][(d, c)] = ut

                def emit_attn(b4):
                    bc = (b4 * LS[0], b4 * LS[1])
                    u = u_t[b4]
                    den = psSC.tile([1, 512], F32, name="den", tag="sc")
                    for d in range(2):
                        w = LS[1 - d]
                        for c in range(CH[d]):
                            nc.tensor.matmul(den[:, d * 256:d * 256 + w], ones_col[:],
                                             u[(d, c)][:, :w],
                                             start=(c == 0), stop=(c == CH[d] - 1))
                    rc = sm.tile([1, 512], F32, name="rc", tag="rc")
                    nc.vector.reciprocal_approx_fast(rc[:, :LS[1]], den[:, :LS[1]])
                    nc.vector.reciprocal_approx_fast(rc[:, 256:256 + LS[0]],
                                                     den[:, 256:256 + LS[0]])
                    # hoisted d0/k0 attention matmuls: PE cover for the
                    # den->recip->rm DVE latency (one tile only -- a fuller
                    # hoist jams the 4-slot psMLP ring)
                    ap00 = psMLP.tile([P, 512], F32, name="attn_ps", tag="mlp")
                    for c in range(CH[0]):
                        nc.tensor.matmul(ap00[:, :LS[1]], eR[b4][0][c][:, 0:128],
                                         u[(0, c)][:, :LS[1]],
                                         start=(c == 0), stop=(c == CH[0] - 1))
                    Rp = psMLP.tile([P, 512], F32, name="R_ps", tag="mlp")
                    for d in range(2):
                        w = LS[1 - d]
                        rm = sm.tile([1, L], BF16, name=f"rm{d}", tag=f"rm{d}")
                        nc.vector.tensor_tensor(rm[:, :w], rc[:, d * 256:d * 256 + w],
                                                mr_t[(b4, 1 - d)][:, :w], op=ALU.mult)
                        nc.tensor.matmul(Rp[:, d * 256:d * 256 + w], ones_row[:], rm[:, :w],
                                         start=True, stop=True)
                    Rs = sm.tile([P, 512], BF16, name="Rs", tag="Rs")
                    nc.any.tensor_copy(Rs[:], Rp[:])
                    for d in range(2):
                        w = LS[1 - d]
                        for k, (k0, k1) in enumerate(EK):
                            kw = k1 - k0
                            if d == 0 and k == 0:
                                ap_ = ap00
                            else:
                                ap_ = psMLP.tile([P, 512], F32, name="attn_ps", tag="mlp")
                                for c in range(CH[d]):
                                    nc.tensor.matmul(ap_[:kw, :w], eR[b4][d][c][:, k0:k1],
                                                     u[(d, c)][:, :w],
                                                     start=(c == 0), stop=(c == CH[d] - 1))
                            nc.vector.tensor_tensor(xsb[1 - d][k][:, bc[1 - d]:bc[1 - d] + w],
                                                    ap_[:kw, :w], Rs[:kw, d * 256:d * 256 + w],
                                                    op=ALU.mult)

                r1 = [[None, None] for _ in range(2)]
                for s in range(2):
                    for m in range(2):
                        r1[s][m] = cmp_.tile([100, GB * L], BF16, name=f"r1{s}{m}", tag=f"r1{s}{m}")

                def emit_compare(s, h, _eT=eT_sb, _x=xsb, _r1=r1, _b0=b0, _LS=LS):
                    # compare L1+L2 for sentence s, half h.  Per-group state is
                    # bound via default args so a deferred call (executed during
                    # the NEXT group's batch phase) still sees this group's tiles.
                    for m, (m0, m1) in enumerate(H2):
                        cp = psMLP.tile([P, 512], F32, name="c1_ps", tag="mlp")
                        for k in range(3):
                            nc.tensor.matmul(cp[:], W1ca_t[k][m][:],
                                             _eT[s][k][:, h * 512:(h + 1) * 512],
                                             start=(k == 0), stop=False)
                        for k in range(3):
                            nc.tensor.matmul(cp[:], W1cb_t[k][m][:],
                                             _x[s][k][:, h * 512:(h + 1) * 512],
                                             start=False, stop=(k == 2))
                        reg = _r1[s][m][:, h * 512:(h + 1) * 512]
                        nc.scalar.activation(reg, cp[:100, :], ACTF.Relu,
                                             bias=b1c_t[:, m:m + 1], scale=1.0)
                    segs = 512 // _LS[s]
                    for m, (m0, m1) in enumerate(H2):
                        cq = psMLP.tile([P, 512], F32, name="c2_ps", tag="mlp")
                        for k2 in range(2):
                            nc.tensor.matmul(cq[:], W2c_t[k2][m][:],
                                             _r1[s][k2][:, h * 512:(h + 1) * 512],
                                             start=(k2 == 0), stop=(k2 == 1))
                        for q in range(segs):
                            b4 = h * segs + q
                            scr = cmp_.tile([100, L], BF16, name="c2scr", tag=f"c2scr{s}")
                            nc.scalar.activation(
                                scr[:, :_LS[s]], cq[:100, q * _LS[s]:(q + 1) * _LS[s]], ACTF.Relu,
                                bias=b2c_t[:, m:m + 1], scale=1.0,
                                accum_out=v_all[s][m][:, _b0 + b4:_b0 + b4 + 1])

                emit_scores(0)
                emit_softmax(0)
                emit_scores(1)
                for fn in pending_cmp:
                    fn()
                pending_cmp = []
                emit_attn(0)
                emit_softmax(1)
                emit_scores(2)
                emit_attn(1)
                emit_softmax(2)
                # halves done after batches 0,1: sentence s half h covers
                # batches [h*512//LS[s], ...); emit compare for halves fully
                # covered by batches 0..1
                for s in range(2):
                    if CH[s] == 2:
                        emit_compare(s, 0)
                emit_scores(3)
                emit_attn(2)
                emit_softmax(3)
                emit_attn(3)
                if g + 1 < NG:
                    eR_next = emit_gathers(g + 1)
                else:
                    eR_next = None
                emit_compare(0, 1 if CH[0] == 2 else 0)
                h1 = 1 if CH[1] == 2 else 0
                if g + 1 < NG:
                    pending_cmp = [lambda f=emit_compare, hh=h1: f(1, hh)]
                else:
                    emit_compare(1, h1)
                eR = eR_next

            # ---------------- aggregate ----------------
            vb = []
            for s in range(2):
                for m in range(2):
                    t = const.tile([100, NB], BF16, name=f"vb{s}{m}", tag=f"vb{s}{m}")
                    nc.vector.tensor_tensor(t[:], v_all[s][m][:], corr[s][m][:], op=ALU.subtract)
                    vb.append(t)
            g1 = []
            for m, (m0, m1) in enumerate(H2):
                gp = psMLP.tile([P, NB], F32, name="g_ps", tag="mlp")
                for k in range(4):
                    nc.tensor.matmul(gp[:], W1g_t[k][m][:], vb[k][:],
                                     start=(k == 0), stop=(k == 3))
                gt = const.tile([100, NB], BF16, name=f"g1{m}", tag=f"g1{m}")
                nc.scalar.activation(gt[:], gp[:100, :], ACTF.Relu, bias=b1g_t[:, m:m + 1], scale=1.0)
                g1.append(gt)
            op = psMLP.tile([2, NB], F32, name="o_ps", tag="mlp")
            for k2 in range(2):
                nc.tensor.matmul(op[:], W2g_t[k2][:], g1[k2][:],
                                 start=(k2 == 0), stop=(k2 == 1))
            osb = const.tile([2, NB], F32, name="osb", tag="osb")
            nc.scalar.activation(osb[:], op[:], ACTF.Identity, bias=b2g_t[:], scale=1.0)
            nc.sync.dma_start(out_d[:].rearrange("b o -> o b"), osb[:])

    nc.compile()
    return nc


def _shard_inputs(inputs, percore, classes):
    import ml_dtypes
    BF = ml_dtypes.bfloat16
    f = np.ascontiguousarray

    emb = np.zeros((VOCAB + 1, EMBED), dtype=BF)
    emb[:VOCAB] = inputs['emb'].astype(BF)

    def padw(w):  # [K, 200] -> [K, 256] with m-chunk m at cols m*128:m*128+100
        out = np.zeros((w.shape[0], 256), dtype=BF)
        out[:, 0:100] = w[:, 0:100].astype(BF)
        out[:, 128:228] = w[:, 100:200].astype(BF)
        return f(out)
    Wb = {k: padw(inputs[k]) for k in ('W1a', 'W2a', 'W1c', 'W2c', 'W1g')}
    Wb['W2g'] = f(inputs['W2g'].astype(BF))
    bias = {k: f(inputs[k].astype(np.float32).reshape(2, 100).T)
            for k in ('b1a', 'b2a', 'b1c', 'b2c', 'b1g')}
    b2g = f(inputs['b2g'].astype(np.float32).reshape(2, 1))

    pos = np.arange(L)
    maps = []
    for cid in range(NCORES):
        idx = np.array(percore[cid], dtype=np.int64)
        cls = classes[cid]
        s = [inputs['s1'][idx].astype(np.int32), inputs['s2'][idx].astype(np.int32)]
        ln = [inputs['len1'][idx].astype(np.int32), inputs['len2'][idx].astype(np.int32)]
        chunks = np.array([[c[0] for c in cls], [c[1] for c in cls]], dtype=np.int32)  # [2, NB]
        sT = np.zeros((128, 4 * NB), dtype=np.int32)
        lmT = np.zeros((128, 4 * NB), dtype=np.float32)
        lenf = np.zeros((1, 2 * NB), dtype=np.int32)
        lmg = np.zeros((1, 2 * NB), dtype=BF)
        for si in range(2):
            valid = pos[None, :] < ln[si][:, None]          # [NB, L]
            sm_ = np.where(valid, s[si], VZERO)
            for c in range(2):
                col = (si * 2 + c) * NB
                sT[:, col:col + NB] = sm_[:, c * 128:(c + 1) * 128].T
                lmT[:, col:col + NB] = np.where(valid[:, c * 128:(c + 1) * 128], 0.0, -30000.0).T
            lenf[0, si * NB:(si + 1) * NB] = ln[si]
            lmg[0, si * NB:(si + 1) * NB] = (128 * chunks[si] - ln[si]).astype(BF)
        maps.append(dict(
            emb=emb, sT=f(sT), lmT=f(lmT), lenf=f(lenf), lmg=f(lmg),
            W1a=Wb['W1a'], W2a=Wb['W2a'], W1c=Wb['W1c'], W2c=Wb['W2c'],
            W1g=Wb['W1g'], W2g=Wb['W2g'],
            b1a=bias['b1a'], b2a=bias['b2a'], b1c=bias['b1c'], b2c=bias['b2c'],
            b1g=bias['b1g'], b2g=b2g,
        ))
    return maps


def kernel(**inputs):
    from concourse.bass_utils import run_bass_kernel_spmd
    len1 = np.asarray(inputs['len1'])
    len2 = np.asarray(inputs['len2'])
    percore, classes, sched = make_schedule(len1, len2)
    key = tuple(sched)
    if key not in _prog_cache:
        _prog_cache[key] = build_program(sched)
        _prog_cache['last'] = (percore, classes, sched)
    nc = _prog_cache[key]
    in_maps = _shard_inputs(inputs, percore, classes)
    res = run_bass_kernel_spmd(nc, in_maps, core_ids=list(range(NCORES)))
    rows = np.concatenate([res.results[c]["out"] for c in range(NCORES)], axis=0)
    perm = np.concatenate([np.array(p, dtype=np.int64) for p in percore])
    out = np.empty((B, 2), dtype=np.float32)
    out[perm] = rows.astype(np.float32)
    return out


# revision 28
# speedup vs baseline: 1.1298x; 1.0152x over previous
"""Trainium2 Bass kernel for DecomposableAttention (B=512, L=256, V=50000, E=300, H=200).

v4: v3 (bf16 PE, 4-batch groups, global-max softmax, zero-row gather,
rank-1 masked-sum correction) + length-class specialization: batches are
host-sorted by (ceil(len1/128), ceil(len2/128)) into classes (1,1), (1,2),
(2,1), (2,2); each group of 4 batches shares a class and only processes the
live 128-position chunks.  Class counts are rounded to multiples of 32 (8
cores x 4 batches) by upgrading leftovers to a superset class, so all cores
run the same program.  The host un-permutes the output rows.
"""
import sys

if '/opt/trn_rl_repo' not in sys.path:
    sys.path.insert(0, '/opt/trn_rl_repo')

import numpy as np

B, L, VOCAB, EMBED, HIDDEN = 512, 256, 50000, 300, 200
NCORES = 8
NB = B // NCORES          # batches per core
GB = 4                    # batches per group
NG = NB // GB             # groups
VZERO = VOCAB             # index of the appended all-zero embedding row

_prog_cache = {}


def make_schedule(len1, len2):
    """Global batch -> per-core permutation + shared group class schedule."""
    ci = np.minimum((len1.astype(np.int64) + 127) // 128, 2)
    cj = np.minimum((len2.astype(np.int64) + 127) // 128, 2)
    buckets = {(1, 1): [], (1, 2): [], (2, 1): [], (2, 2): []}
    for idx in range(len(len1)):
        buckets[(int(ci[idx]), int(cj[idx]))].append(idx)
    unit = NCORES * GB
    for src, dst in [((1, 1), (1, 2)), ((2, 1), (2, 2)), ((1, 2), (2, 2))]:
        keep = len(buckets[src]) - (len(buckets[src]) % unit)
        buckets[dst] = buckets[src][keep:] + buckets[dst]
        buckets[src] = buckets[src][:keep]
    assert len(buckets[(2, 2)]) % unit == 0
    percore = [[] for _ in range(NCORES)]
    classes = [[] for _ in range(NCORES)]   # class per batch slot
    sched = []
    for c in [(1, 1), (1, 2), (2, 1), (2, 2)]:
        lst = buckets[c]
        n = len(lst) // NCORES
        for core in range(NCORES):
            percore[core] += lst[core * n:(core + 1) * n]
            classes[core] += [c] * n
        sched += [c] * (n // GB)
    assert len(sched) == NG
    return percore, classes, sched


def build_program(sched):
    import concourse.bass as bass
    import concourse.bass_isa as bass_isa
    import concourse.bacc as bacc
    import concourse.tile as tile
    import concourse.mybir as mybir
    from concourse.masks import make_identity

    F32 = mybir.dt.float32
    BF16 = mybir.dt.bfloat16
    I32 = mybir.dt.int32
    AX = mybir.AxisListType
    ALU = mybir.AluOpType
    ACTF = mybir.ActivationFunctionType
    P = 128
    EK = [(0, 128), (128, 256), (256, 300)]     # E contraction chunks
    H2 = [(0, 100), (100, 200)]                 # H chunks of 100

    nc = bacc.Bacc("TRN2", num_devices=NCORES)

    emb_d = nc.dram_tensor("emb", [VOCAB + 1, EMBED], BF16, kind="ExternalInput")
    sT_d = nc.dram_tensor("sT", [P, 4 * NB], I32, kind="ExternalInput")
    lmT_d = nc.dram_tensor("lmT", [P, 4 * NB], F32, kind="ExternalInput")
    lenf_d = nc.dram_tensor("lenf", [1, 2 * NB], I32, kind="ExternalInput")
    lmg_d = nc.dram_tensor("lmg", [1, 2 * NB], BF16, kind="ExternalInput")
    # weight matrices are host-padded to 128-column m-blocks so the
    # compiler enables Fast Weight Load (needs exactly 128 weight columns)
    W1a_d = nc.dram_tensor("W1a", [EMBED, 256], BF16, kind="ExternalInput")
    W2a_d = nc.dram_tensor("W2a", [HIDDEN, 256], BF16, kind="ExternalInput")
    W1c_d = nc.dram_tensor("W1c", [2 * EMBED, 256], BF16, kind="ExternalInput")
    W2c_d = nc.dram_tensor("W2c", [HIDDEN, 256], BF16, kind="ExternalInput")
    W1g_d = nc.dram_tensor("W1g", [2 * HIDDEN, 256], BF16, kind="ExternalInput")
    W2g_d = nc.dram_tensor("W2g", [HIDDEN, 2], BF16, kind="ExternalInput")
    b1a_d = nc.dram_tensor("b1a", [100, 2], F32, kind="ExternalInput")
    b2a_d = nc.dram_tensor("b2a", [100, 2], F32, kind="ExternalInput")
    b1c_d = nc.dram_tensor("b1c", [100, 2], F32, kind="ExternalInput")
    b2c_d = nc.dram_tensor("b2c", [100, 2], F32, kind="ExternalInput")
    b1g_d = nc.dram_tensor("b1g", [100, 2], F32, kind="ExternalInput")
    b2g_d = nc.dram_tensor("b2g", [2, 1], F32, kind="ExternalInput")
    out_d = nc.dram_tensor("out", [NB, 2], F32, kind="ExternalOutput")

    with tile.TileContext(nc) as tc:
        import contextlib
        ctx = contextlib.ExitStack()
        with ctx:
            const = ctx.enter_context(tc.tile_pool(name="const", bufs=1))
            gat = ctx.enter_context(tc.tile_pool(name="gat", bufs=2))
            eTp = ctx.enter_context(tc.tile_pool(name="eTp", bufs=2))
            hp = ctx.enter_context(tc.tile_pool(name="hp", bufs=2))
            up = ctx.enter_context(tc.tile_pool(name="up", bufs=2))
            sm = ctx.enter_context(tc.tile_pool(name="sm", bufs=2))
            cmp_ = ctx.enter_context(tc.tile_pool(name="cmp", bufs=2))
            psMLP = ctx.enter_context(tc.tile_pool(name="psMLP", bufs=4, space="PSUM"))
            psSC = ctx.enter_context(tc.tile_pool(name="psSC", bufs=4, space="PSUM"))

            # inputs needed by the first group's gathers -- DMA'd first
            sT_t = const.tile([P, 4 * NB], I32)
            nc.sync.dma_start(sT_t[:], sT_d[:])
            lmT_t = const.tile([P, 4 * NB], F32)
            nc.sync.dma_start(lmT_t[:], lmT_d[:])
            lenf_t = const.tile([1, 2 * NB], I32)
            nc.sync.dma_start(lenf_t[:], lenf_d[:])
            lmg_t = const.tile([1, 2 * NB], BF16)
            nc.sync.dma_start(lmg_t[:], lmg_d[:])

            # ---------------- constants ----------------
            ident_f = const.tile([P, P], F32)
            make_identity(nc, ident_f[:])
            ident = const.tile([P, P], BF16)
            nc.vector.tensor_copy(ident[:], ident_f[:])

            ones_col = const.tile([P, 1], BF16)
            nc.vector.memset(ones_col[:], 1.0)
            ones_row = const.tile([1, P], BF16)
            nc.vector.memset(ones_row[:], 1.0)

            iota_row = const.tile([1, L], I32)
            nc.gpsimd.iota(iota_row[:], pattern=[[1, L]], base=0, channel_multiplier=0)

            # weights (stationary tiles, bf16)
            def wtiles(dram, rows, nm):
                # per-(k-chunk, m-chunk) compact [kw, 128] tiles so the weight
                # AP is contiguous (FWL-safe)
                ts = []
                for i, (k0, k1) in enumerate(rows):
                    pair = []
                    for m in range(2):
                        t = const.tile([k1 - k0, 128], BF16, name=f"{nm}{i}{m}", tag=f"{nm}{i}{m}")
                        nc.sync.dma_start(t[:], dram[k0:k1, m * 128:(m + 1) * 128])
                        pair.append(t)
                    ts.append(pair)
                return ts
            W1a_t = wtiles(W1a_d, EK, "W1a")
            W2a_t = wtiles(W2a_d, H2, "W2a")
            W1ca_t = wtiles(W1c_d, EK, "W1ca")
            W1cb_t = wtiles(W1c_d, [(EMBED + k0, EMBED + k1) for k0, k1 in EK], "W1cb")
            W2c_t = wtiles(W2c_d, H2, "W2c")
            W1g_t = wtiles(W1g_d, [(i * 100, (i + 1) * 100) for i in range(4)], "W1g")
            W2g_t = [const.tile([100, 2], BF16, name=f"W2g{i}", tag=f"W2g{i}") for i in range(2)]
            for i, (k0, k1) in enumerate(H2):
                nc.sync.dma_start(W2g_t[i][:], W2g_d[k0:k1, :])

            def bias_tile(d, nm):
                t = const.tile([d.shape[0], d.shape[1]], F32, name=nm, tag=nm)
                nc.sync.dma_start(t[:], d[:])
                return t
            b1a_t = bias_tile(b1a_d, "b1a")
            b2a_t = bias_tile(b2a_d, "b2a")
            b1c_t = bias_tile(b1c_d, "b1c")
            b2c_t = bias_tile(b2c_d, "b2c")
            b1g_t = bias_tile(b1g_d, "b1g")
            b2g_t = bias_tile(b2g_d, "b2g")

            v_all = [[const.tile([100, NB], F32, name=f"v{s}{m}", tag=f"v{s}{m}")
                      for m in range(2)] for s in range(2)]

            # ---- c0 (compare output at an all-zero input column) + corr ----
            rb1c = const.tile([100, 2], BF16)
            nc.scalar.activation(rb1c[:], b1c_t[:], ACTF.Relu, bias=0.0, scale=1.0)
            c0T = []
            for m, (m0, m1) in enumerate(H2):
                cps = psMLP.tile([P, 1], F32, name="c0_ps", tag="mlp")
                for k in range(2):
                    nc.tensor.matmul(cps[:], W2c_t[k][m][:], rb1c[:, k:k + 1],
                                     start=(k == 0), stop=(k == 1))
                c0m = const.tile([100, 1], F32, name=f"c0{m}", tag=f"c0{m}")
                nc.scalar.activation(c0m[:], cps[:100, :], ACTF.Relu, bias=b2c_t[:, m:m + 1], scale=1.0)
                tps = psSC.tile([1, P], F32, name="c0T_ps", tag="sc")
                nc.tensor.transpose(tps[:, :100], c0m[:], ident_f[:100, :100])
                c0t = const.tile([1, 100], BF16, name=f"c0T{m}", tag=f"c0T{m}")
                nc.vector.tensor_copy(c0t[:], tps[:, :100])
                c0T.append(c0t)
            corr = [[None, None], [None, None]]
            for s in range(2):
                for m in range(2):
                    cps = psMLP.tile([100, NB], F32, name="corr_ps", tag="mlp")
                    nc.tensor.matmul(cps[:], c0T[m][:], lmg_t[:, s * NB:(s + 1) * NB],
                                     start=True, stop=True)
                    ct = const.tile([100, NB], F32, name=f"corr{s}{m}", tag=f"corr{s}{m}")
                    nc.vector.tensor_copy(ct[:], cps[:])
                    corr[s][m] = ct

            def scol(s, c):  # column base in sT/lmT for (sentence, L-chunk)
                return (s * 2 + c) * NB

            def emit_gathers(g):
                b0 = g * GB
                I, J = sched[g]
                CH = (I, J)
                eRg = [[[None, None] for _ in range(2)] for _ in range(GB)]
                for s in range(2):
                    for b4 in range(GB):
                        for c in range(CH[s]):
                            t = gat.tile([P, EMBED], BF16, name=f"eR{b4}{s}{c}", tag=f"eR{b4}{s}{c}")
                            nc.gpsimd.indirect_dma_start(
                                out=t[:], out_offset=None, in_=emb_d[:],
                                in_offset=bass.IndirectOffsetOnAxis(
                                    ap=sT_t[:, scol(s, c) + b0 + b4: scol(s, c) + b0 + b4 + 1],
                                    axis=0),
                            )
                            eRg[b4][s][c] = t
                return eRg

            # ---------------- group loop ----------------
            pending_cmp = []
            eR = emit_gathers(0)
            for g in range(NG):
                b0 = g * GB
                I, J = sched[g]
                CH = (I, J)                    # chunks per sentence
                LS = (128 * I, 128 * J)        # live positions per sentence

                eT_sb = [[None] * 3 for _ in range(2)]
                for s in range(2):
                    for k, (k0, k1) in enumerate(EK):
                        eT_sb[s][k] = eTp.tile([k1 - k0, GB * L], BF16,
                                               name=f"eT{s}{k}", tag=f"eT{s}{k}")

                def emit_tr(s, k, h):
                    # one PSUM tile = 4 transposed [128,128] quarters = 512 cols
                    k0, k1 = EK[k]
                    kw = k1 - k0
                    nch = CH[s]
                    tp = psSC.tile([P, 512], BF16, name="tr_ps", tag="sc")
                    for q in range(4):
                        pos = h * 4 + q
                        b4, c = divmod(pos, nch)
                        nc.tensor.transpose(tp[:kw, q * P:(q + 1) * P],
                                            eR[b4][s][c][:, k0:k1], ident[:])
                    nc.vector.tensor_copy(eT_sb[s][k][:, h * 512:(h + 1) * 512], tp[:kw, :])

                # --- attend L1 (m=0 pass interleaves the transposes) ---
                ha = [[None, None] for _ in range(2)]
                hT = [[None, None] for _ in range(2)]
                for s in range(2):
                    nh = CH[s]                 # halves of 512 cols for this sentence
                    for m, (m0, m1) in enumerate(H2):
                        for h in range(nh):
                            if m == 0:
                                emit_tr(s, 0, h)
                            pp = psMLP.tile([P, 512], F32, name="a1_ps", tag="mlp")
                            for k in range(3):
                                if m == 0 and k + 1 < 3:
                                    emit_tr(s, k + 1, h)
                                nc.tensor.matmul(pp[:], W1a_t[k][m][:],
                                                 eT_sb[s][k][:, h * 512:(h + 1) * 512],
                                                 start=(k == 0), stop=(k == 2))
                            if ha[s][m] is None:
                                ha[s][m] = hp.tile([100, GB * L], BF16, name=f"ha{s}{m}", tag=f"ha{s}{m}")
                            nc.scalar.activation(ha[s][m][:, h * 512:(h + 1) * 512], pp[:100, :],
                                                 ACTF.Relu, bias=b1a_t[:, m:m + 1], scale=1.0)
                # --- attend L2 ---
                for s in range(2):
                    nh = CH[s]
                    for m, (m0, m1) in enumerate(H2):
                        for h in range(nh):
                            qp = psMLP.tile([P, 512], F32, name="a2_ps", tag="mlp")
                            for k2 in range(2):
                                nc.tensor.matmul(qp[:], W2a_t[k2][m][:],
                                                 ha[s][k2][:, h * 512:(h + 1) * 512],
                                                 start=(k2 == 0), stop=(k2 == 1))
                            if hT[s][m] is None:
                                hT[s][m] = hp.tile([100, GB * L], BF16, name=f"hT{s}{m}", tag=f"hT{s}{m}")
                            nc.scalar.activation(hT[s][m][:, h * 512:(h + 1) * 512], qp[:100, :],
                                                 ACTF.Relu, bias=b2a_t[:, m:m + 1], scale=1.0)

                # --- per-batch phase ---
                xsb = [[None] * 3 for _ in range(2)]
                for s in range(2):
                    for k, (k0, k1) in enumerate(EK):
                        xsb[s][k] = eTp.tile([k1 - k0, GB * L], BF16, name=f"x{s}{k}", tag=f"x{s}{k}")

                mr_t = {}
                for b4 in range(GB):
                    for si in range(2):
                        mr = sm.tile([1, L], F32, name=f"mr{b4}{si}", tag=f"mr{b4}{si}")
                        nc.vector.tensor_tensor(
                            mr[:], iota_row[:],
                            lenf_t[:, si * NB + b0 + b4: si * NB + b0 + b4 + 1].to_broadcast([1, L]),
                            op=ALU.is_lt)
                        mr_t[(b4, si)] = mr

                pe_t, pet_t, u_t, bias_t = {}, {}, {}, {}

                def emit_scores(b4):
                    bc = (b4 * LS[0], b4 * LS[1])
                    pe = psSC.tile([P, 512], F32, name="pe", tag="sc")
                    pet = psSC.tile([P, 512], F32, name="pet", tag="sc")
                    for ic in range(I):
                        for m in range(2):
                            nc.tensor.matmul(pe[:, ic * LS[1]:(ic + 1) * LS[1]],
                                             hT[0][m][:, bc[0] + ic * P: bc[0] + (ic + 1) * P],
                                             hT[1][m][:, bc[1]:bc[1] + LS[1]],
                                             start=(m == 0), stop=(m == 1))
                    for jc in range(J):
                        for m in range(2):
                            nc.tensor.matmul(pet[:, jc * LS[0]:(jc + 1) * LS[0]],
                                             hT[1][m][:, bc[1] + jc * P: bc[1] + (jc + 1) * P],
                                             hT[0][m][:, bc[0]:bc[0] + LS[0]],
                                             start=(m == 0), stop=(m == 1))
                    pe_t[b4], pet_t[b4] = pe, pet

                def emit_softmax(b4):
                    b = b0 + b4
                    pe, pet = pe_t[b4], pet_t[b4]
                    mx = sm.tile([P, 1], F32, name="mx", tag="mx")
                    nc.vector.tensor_reduce(mx[:], pe[:, :I * LS[1]], axis=AX.X, op=ALU.max)
                    Gb = sm.tile([P, 1], F32, name="Gb", tag="Gb")
                    nc.gpsimd.partition_all_reduce(Gb[:], mx[:], channels=P,
                                                   reduce_op=bass_isa.ReduceOp.max)
                    bias_t[b4] = {}
                    for d in range(2):
                        for c in range(CH[d]):
                            bt = sm.tile([P, 1], F32, name=f"bx{d}{c}", tag=f"bx{d}{c}")
                            nc.vector.tensor_tensor(
                                bt[:], lmT_t[:, scol(d, c) + b: scol(d, c) + b + 1],
                                Gb[:], op=ALU.subtract)
                            bias_t[b4][(d, c)] = bt
                    u_t[b4] = {}
                    for d, src in ((0, pe), (1, pet)):
                        w = LS[1 - d]
                        for c in range(CH[d]):
                            ut = up.tile([P, L], BF16, name=f"u{b4}{d}{c}", tag=f"u{b4}{d}{c}")
                            nc.scalar.activation(ut[:, :w], src[:, c * w:(c + 1) * w], ACTF.Exp,
                                                 bias=bias_t[b4][(d, c)][:], scale=1.0)
                            u_t[b4][(d, c)] = ut

                def emit_attn(b4):
                    bc = (b4 * LS[0], b4 * LS[1])
                    u = u_t[b4]
                    den = psSC.tile([1, 512], F32, name="den", tag="sc")
                    for d in range(2):
                        w = LS[1 - d]
                        for c in range(CH[d]):
                            nc.tensor.matmul(den[:, d * 256:d * 256 + w], ones_col[:],
                                             u[(d, c)][:, :w],
                                             start=(c == 0), stop=(c == CH[d] - 1))
                    rc = sm.tile([1, 512], F32, name="rc", tag="rc")
                    nc.vector.reciprocal_approx_fast(rc[:, :LS[1]], den[:, :LS[1]])
                    nc.vector.reciprocal_approx_fast(rc[:, 256:256 + LS[0]],
                                                     den[:, 256:256 + LS[0]])
                    Rp = psMLP.tile([P, 512], F32, name="R_ps", tag="mlp")
                    for d in range(2):
                        w = LS[1 - d]
                        rm = sm.tile([1, L], BF16, name=f"rm{d}", tag=f"rm{d}")
                        nc.vector.tensor_tensor(rm[:, :w], rc[:, d * 256:d * 256 + w],
                                                mr_t[(b4, 1 - d)][:, :w], op=ALU.mult)
                        nc.tensor.matmul(Rp[:, d * 256:d * 256 + w], ones_row[:], rm[:, :w],
                                         start=True, stop=True)
                    Rs = sm.tile([P, 512], BF16, name="Rs", tag="Rs")
                    nc.any.tensor_copy(Rs[:], Rp[:])
                    for d in range(2):
                        w = LS[1 - d]
                        for k, (k0, k1) in enumerate(EK):
                            kw = k1 - k0
                            ap_ = psMLP.tile([P, 512], F32, name="attn_ps", tag="mlp")
                            for c in range(CH[d]):
                                nc.tensor.matmul(ap_[:kw, :w], eR[b4][d][c][:, k0:k1],
                                                 u[(d, c)][:, :w],
                                                 start=(c == 0), stop=(c == CH[d] - 1))
                            nc.vector.tensor_tensor(xsb[1 - d][k][:, bc[1 - d]:bc[1 - d] + w],
                                                    ap_[:kw, :w], Rs[:kw, d * 256:d * 256 + w],
                                                    op=ALU.mult)

                r1 = [[None, None] for _ in range(2)]
                for s in range(2):
                    for m in range(2):
                        r1[s][m] = cmp_.tile([100, GB * L], BF16, name=f"r1{s}{m}", tag=f"r1{s}{m}")

                def emit_compare(s, h, _eT=eT_sb, _x=xsb, _r1=r1, _b0=b0, _LS=LS):
                    # compare L1+L2 for sentence s, half h.  Per-group state is
                    # bound via default args so a deferred call (executed during
                    # the NEXT group's batch phase) still sees this group's tiles.
                    for m, (m0, m1) in enumerate(H2):
                        cp = psMLP.tile([P, 512], F32, name="c1_ps", tag="mlp")
                        for k in range(3):
                            nc.tensor.matmul(cp[:], W1ca_t[k][m][:],
                                             _eT[s][k][:, h * 512:(h + 1) * 512],
                                             start=(k == 0), stop=False)
                        for k in range(3):
                            nc.tensor.matmul(cp[:], W1cb_t[k][m][:],
                                             _x[s][k][:, h * 512:(h + 1) * 512],
                                             start=False, stop=(k == 2))
                        reg = _r1[s][m][:, h * 512:(h + 1) * 512]
                        nc.scalar.activation(reg, cp[:100, :], ACTF.Relu,
                                             bias=b1c_t[:, m:m + 1], scale=1.0)
                    segs = 512 // _LS[s]
                    for m, (m0, m1) in enumerate(H2):
                        cq = psMLP.tile([P, 512], F32, name="c2_ps", tag="mlp")
                        for k2 in range(2):
                            nc.tensor.matmul(cq[:], W2c_t[k2][m][:],
                                             _r1[s][k2][:, h * 512:(h + 1) * 512],
                                             start=(k2 == 0), stop=(k2 == 1))
                        for q in range(segs):
                            b4 = h * segs + q
                            scr = cmp_.tile([100, L], BF16, name="c2scr", tag=f"c2scr{s}")
                            nc.scalar.activation(
                                scr[:, :_LS[s]], cq[:100, q * _LS[s]:(q + 1) * _LS[s]], ACTF.Relu,
                                bias=b2c_t[:, m:m + 1], scale=1.0,
                                accum_out=v_all[s][m][:, _b0 + b4:_b0 + b4 + 1])

                emit_scores(0)
                emit_softmax(0)
                emit_scores(1)
                for fn in pending_cmp:
                    fn()
                pending_cmp = []
                emit_attn(0)
                emit_softmax(1)
                emit_scores(2)
                emit_attn(1)
                emit_softmax(2)
                # halves done after batches 0,1: sentence s half h covers
                # batches [h*512//LS[s], ...); emit compare for halves fully
                # covered by batches 0..1
                for s in range(2):
                    if CH[s] == 2:
                        emit_compare(s, 0)
                emit_scores(3)
                emit_attn(2)
                emit_softmax(3)
                emit_attn(3)
                if g + 1 < NG:
                    eR_next = emit_gathers(g + 1)
                else:
                    eR_next = None
                emit_compare(0, 1 if CH[0] == 2 else 0)
                h1 = 1 if CH[1] == 2 else 0
                if g + 1 < NG:
                    pending_cmp = [lambda f=emit_compare, hh=h1: f(1, hh)]
                else:
                    emit_compare(1, h1)
                eR = eR_next

            # ---------------- aggregate ----------------
            vb = []
            for s in range(2):
                for m in range(2):
                    t = const.tile([100, NB], BF16, name=f"vb{s}{m}", tag=f"vb{s}{m}")
                    nc.vector.tensor_tensor(t[:], v_all[s][m][:], corr[s][m][:], op=ALU.subtract)
                    vb.append(t)
            g1 = []
            for m, (m0, m1) in enumerate(H2):
                gp = psMLP.tile([P, NB], F32, name="g_ps", tag="mlp")
                for k in range(4):
                    nc.tensor.matmul(gp[:], W1g_t[k][m][:], vb[k][:],
                                     start=(k == 0), stop=(k == 3))
                gt = const.tile([100, NB], BF16, name=f"g1{m}", tag=f"g1{m}")
                nc.scalar.activation(gt[:], gp[:100, :], ACTF.Relu, bias=b1g_t[:, m:m + 1], scale=1.0)
                g1.append(gt)
            op = psMLP.tile([2, NB], F32, name="o_ps", tag="mlp")
            for k2 in range(2):
                nc.tensor.matmul(op[:], W2g_t[k2][:], g1[k2][:],
                                 start=(k2 == 0), stop=(k2 == 1))
            osb = const.tile([2, NB], F32, name="osb", tag="osb")
            nc.scalar.activation(osb[:], op[:], ACTF.Identity, bias=b2g_t[:], scale=1.0)
            nc.sync.dma_start(out_d[:].rearrange("b o -> o b"), osb[:])

    nc.compile()
    return nc


def _shard_inputs(inputs, percore, classes):
    import ml_dtypes
    BF = ml_dtypes.bfloat16
    f = np.ascontiguousarray

    emb = np.zeros((VOCAB + 1, EMBED), dtype=BF)
    emb[:VOCAB] = inputs['emb'].astype(BF)

    def padw(w):  # [K, 200] -> [K, 256] with m-chunk m at cols m*128:m*128+100
        out = np.zeros((w.shape[0], 256), dtype=BF)
        out[:, 0:100] = w[:, 0:100].astype(BF)
        out[:, 128:228] = w[:, 100:200].astype(BF)
        return f(out)
    Wb = {k: padw(inputs[k]) for k in ('W1a', 'W2a', 'W1c', 'W2c', 'W1g')}
    Wb['W2g'] = f(inputs['W2g'].astype(BF))
    bias = {k: f(inputs[k].astype(np.float32).reshape(2, 100).T)
            for k in ('b1a', 'b2a', 'b1c', 'b2c', 'b1g')}
    b2g = f(inputs['b2g'].astype(np.float32).reshape(2, 1))

    pos = np.arange(L)
    maps = []
    for cid in range(NCORES):
        idx = np.array(percore[cid], dtype=np.int64)
        cls = classes[cid]
        s = [inputs['s1'][idx].astype(np.int32), inputs['s2'][idx].astype(np.int32)]
        ln = [inputs['len1'][idx].astype(np.int32), inputs['len2'][idx].astype(np.int32)]
        chunks = np.array([[c[0] for c in cls], [c[1] for c in cls]], dtype=np.int32)  # [2, NB]
        sT = np.zeros((128, 4 * NB), dtype=np.int32)
        lmT = np.zeros((128, 4 * NB), dtype=np.float32)
        lenf = np.zeros((1, 2 * NB), dtype=np.int32)
        lmg = np.zeros((1, 2 * NB), dtype=BF)
        for si in range(2):
            valid = pos[None, :] < ln[si][:, None]          # [NB, L]
            sm_ = np.where(valid, s[si], VZERO)
            for c in range(2):
                col = (si * 2 + c) * NB
                sT[:, col:col + NB] = sm_[:, c * 128:(c + 1) * 128].T
                lmT[:, col:col + NB] = np.where(valid[:, c * 128:(c + 1) * 128], 0.0, -30000.0).T
            lenf[0, si * NB:(si + 1) * NB] = ln[si]
            lmg[0, si * NB:(si + 1) * NB] = (128 * chunks[si] - ln[si]).astype(BF)
        maps.append(dict(
            emb=emb, sT=f(sT), lmT=f(lmT), lenf=f(lenf), lmg=f(lmg),
            W1a=Wb['W1a'], W2a=Wb['W2a'], W1c=Wb['W1c'], W2c=Wb['W2c'],
            W1g=Wb['W1g'], W2g=Wb['W2g'],
            b1a=bias['b1a'], b2a=bias['b2a'], b1c=bias['b1c'], b2c=bias['b2c'],
            b1g=bias['b1g'], b2g=b2g,
        ))
    return maps


def kernel(**inputs):
    from concourse.bass_utils import run_bass_kernel_spmd
    len1 = np.asarray(inputs['len1'])
    len2 = np.asarray(inputs['len2'])
    percore, classes, sched = make_schedule(len1, len2)
    key = tuple(sched)
    if key not in _prog_cache:
        _prog_cache[key] = build_program(sched)
        _prog_cache['last'] = (percore, classes, sched)
    nc = _prog_cache[key]
    in_maps = _shard_inputs(inputs, percore, classes)
    res = run_bass_kernel_spmd(nc, in_maps, core_ids=list(range(NCORES)))
    rows = np.concatenate([res.results[c]["out"] for c in range(NCORES)], axis=0)
    perm = np.concatenate([np.array(p, dtype=np.int64) for p in percore])
    out = np.empty((B, 2), dtype=np.float32)
    out[perm] = rows.astype(np.float32)
    return out
